# revision 40
# baseline (speedup 1.0000x reference)
"""Trainium2 Bass kernel for AngelLoss (center loss + angular loss).

loss = 0.5*sum((feat - centers[y])^2)/B
     + sum_offdiag((c_i.c_j/(|c_i||c_j|) - ct)^2) / (0.5*C*(C-1))

Sharding (8 NeuronCores, data-parallel over batch):
  - center term, gather-free:  sum||f||^2 - 2*sum_c c_c.S_c + sum_c n_c||c_c||^2
    where S_c = sum of feat rows with label c.  Host buckets each core's
    8192 rows into 8 class-banks (125 classes each, greedy-balanced), pads
    each bank's rows to 9x128 and ships them fp8 along with fp8 onehots.
    S for two banks accumulates in one 2-bank PSUM tile via 2x9
    onehot^T @ feat fp8 matmuls and drains with one DVE multiply +
    free-axis reduce against the resident bf16 centers.  sum||f||^2
    splits 7/2 across ScalarE (square+accum) and DVE (mult+reduce).
  - angular term via the Frobenius identity (N = row-normalized centers):
      sum_ij (sim-ct)^2 = ||N^T N||_F^2 - 2ct ||sum_i N_i||^2 + C^2 ct^2
    computed redundantly on every core from the resident fp8 normalized
    table with 2x16 accumulating matmuls into 2-bank PSUM Gram halves.
  - per-core [1,32] partial sums are combined on the host.
"""

from contextlib import ExitStack

import ml_dtypes
import numpy as np

import concourse.bass as bass
import concourse.tile as tile
from concourse import bacc, mybir
from concourse.bass import ds, ts
from concourse.bass_utils import run_bass_kernel_spmd

N_CORES = 8
B, C, D = 65536, 1000, 512
BS = B // N_CORES  # 8192 rows per core
NB = 8  # class banks
CPB = C // NB  # 125 classes per bank
SUB = 9  # 128-row subtiles per bank group (1152 slots >= ~1024+slack rows)
GROUP = 128 * SUB  # 1152
PR = NB * GROUP  # 9216 padded rows per core
NSC = 7  # feat slots squared on ScalarE; the rest (SUB-NSC) go to DVE

# ct = 2*radius(C-1)^2 - 1 from the reference, evaluated in f64, cast f32.
CT = float(np.float32(-0.0010010010010047532))

_F32 = mybir.dt.float32
_BF16 = mybir.dt.bfloat16
_FP8 = mybir.dt.float8e4

_NC_CACHE = {}


def _build_body(ctx, tc, feat, cnt, oh, cbf, nbf, out):
    nc = tc.nc
    AF = mybir.ActivationFunctionType

    const = ctx.enter_context(tc.tile_pool(name="const", bufs=1))
    pnrm = ctx.enter_context(tc.tile_pool(name="nrm", bufs=2))
    pfeat = ctx.enter_context(tc.tile_pool(name="feat", bufs=4))
    psq = ctx.enter_context(tc.tile_pool(name="sq", bufs=2))
    pdscr = ctx.enter_context(tc.tile_pool(name="dscr", bufs=2))
    pG = ctx.enter_context(tc.tile_pool(name="G", bufs=1, space="PSUM"))
    pS = ctx.enter_context(tc.tile_pool(name="S", bufs=2, space="PSUM"))

    # scalar queue: onehots + counts; gpsimd queue: the two tables;
    # sync queue: the feat stream (nothing else competes with it)
    oht = const.tile([128, NB * SUB, 128], _FP8)
    ct_all = const.tile([128, NB, D], _FP8)
    nt_all = const.tile([128, NB, D], _FP8)
    cntt = const.tile([128, 16], _F32)
    # everything but the feat stream issues on the otherwise-idle gpsimd
    # ring, ordered by when the consumer needs it
    nc.gpsimd.dma_start(oht[:, ds(0, 2 * SUB), :], oh[:, ds(0, 2 * SUB), :])
    nc.gpsimd.dma_start(ct_all[:], cbf.rearrange("(p s) d -> p s d", p=128))
    nc.gpsimd.dma_start(oht[:, ds(2 * SUB, 2 * SUB), :], oh[:, ds(2 * SUB, 2 * SUB), :])
    nc.gpsimd.dma_start(nt_all[:], nbf.rearrange("(p s) d -> p s d", p=128))
    nc.gpsimd.dma_start(oht[:, ds(4 * SUB, 2 * SUB), :], oh[:, ds(4 * SUB, 2 * SUB), :])
    nc.gpsimd.dma_start(oht[:, ds(6 * SUB, 2 * SUB), :], oh[:, ds(6 * SUB, 2 * SUB), :])
    nc.gpsimd.dma_start(cntt[:], cnt[:, :])

    ones = const.tile([128, 1], _F32)
    nc.vector.memset(ones[:], 1.0)
    onesp2 = const.tile([128, 2, 1], _FP8)
    nc.vector.memset(onesp2[:], 1.0)
    # staging cols: 0-7 scalar sum(f^2); 8 counts.|c|^2; 9,15 ||G||^2
    # halves; 10-13 +cross per bank pair (host applies -2); 14 ||colsum||^2;
    # 16-23 vector sum(f^2).
    staging = const.tile([128, 32], _F32)
    nc.vector.memset(staging[:], 0.0)

    # --- center-loss main loop; S PSUM tile covers two banks ---
    for g in range(NB):
        if g % 2 == 0:
            st = pS.tile([128, 2, D], _F32, tag="S")
        ft = pfeat.tile([128, SUB, D], _FP8, tag="ft")
        nc.sync.dma_start(
            ft[:], feat[ds(g * GROUP, GROUP), :].rearrange("(p s) d -> p s d", p=128)
        )
        sqs = psq.tile([128, NSC, D], _FP8, tag="sqs")
        nc.scalar.activation(
            sqs[:], ft[:, :NSC, :], AF.Square, accum_out=staging[:, g : g + 1]
        )
        sqv = psq.tile([128, SUB - NSC, D], _FP8, tag="sqv")
        nc.vector.tensor_tensor(
            out=sqv[:],
            in0=ft[:, NSC:, :],
            in1=ft[:, NSC:, :],
            op=mybir.AluOpType.mult,
        )
        nc.vector.tensor_reduce(
            out=staging[:, 16 + g : 17 + g],
            in_=sqv[:],
            axis=mybir.AxisListType.XY,
            op=mybir.AluOpType.add,
        )
        for sp in range(0, SUB - 1, 2):
            nc.tensor.matmul(
                st[:, g % 2, :],
                oht[:, ds(g * SUB + sp, 2), :],
                ft[:, ds(sp, 2), :],
                start=(sp == 0),
                stop=False,
                perf_mode=mybir.MatmulPerfMode.DoubleRow,
            )
        nc.tensor.matmul(
            st[:, g % 2, :],
            oht[:, g * SUB + SUB - 1, :],
            ft[:, SUB - 1, :],
            start=False,
            stop=True,
        )
        if g % 2 == 1:
            dscr = pdscr.tile([CPB, 2, D], _F32, tag="dscr")
            nc.vector.tensor_tensor(
                out=dscr[:],
                in0=st[:CPB, :, :],
                in1=ct_all[:CPB, ds(g - 1, 2), :],
                op=mybir.AluOpType.mult,
            )
            nc.vector.tensor_reduce(
                out=staging[:CPB, 10 + g // 2 : 11 + g // 2],
                in_=dscr[:],
                axis=mybir.AxisListType.XY,
                op=mybir.AluOpType.add,
            )
        if g in (2, 4):
            # angular Gram (two 2-bank passes) slots into the Tensor
            # stream here: the normalized table is resident and the
            # scatter stream has slack
            kis = (0, 1) if g == 2 else (2, 3)
            Gt = pG.tile([128, 2, D], _F32, tag="G")
            for jp in range(0, NB, 2):
                for kx, ki in enumerate(kis):
                    nc.tensor.matmul(
                        Gt[:, kx, :],
                        nt_all[:CPB, ds(jp, 2), ts(ki, 128)],
                        nt_all[:CPB, ds(jp, 2), :],
                        start=(jp == 0),
                        stop=(jp == NB - 2),
                        perf_mode=mybir.MatmulPerfMode.DoubleRow,
                    )
            col = 9 if g == 2 else 15
            gsq = pnrm.tile([128, 2, D], _F32, tag="gsq")
            nc.scalar.activation(
                gsq[:], Gt[:], AF.Square, accum_out=staging[:, col : col + 1]
            )
        if g == 3:
            # colsum of the normalized table on the idle cs PSUM bank
            csf = pG.tile([1, D], _F32, tag="cs")
            for j in range(NB):
                nc.tensor.matmul(
                    csf[0:1, :],
                    onesp2[:CPB, 0, :],
                    nt_all[:CPB, j, :],
                    start=(j == 0),
                    stop=(j == NB - 1),
                )
            css = pnrm.tile([1, D], _F32, tag="css")
            nc.scalar.activation(
                css[0:1, :], csf[0:1, :], AF.Square, accum_out=staging[0:1, 14:15]
            )
        if g == 4:
            # counts . |c|^2 (norms^2 host-packed beside the counts)
            cscr = pnrm.tile([CPB, NB], _F32, tag="cscr")
            nc.vector.tensor_tensor(
                out=cscr[:],
                in0=cntt[:CPB, 0:NB],
                in1=cntt[:CPB, NB:16],
                op=mybir.AluOpType.mult,
            )
            nc.vector.tensor_reduce(
                out=staging[:CPB, 8:9],
                in_=cscr[:],
                axis=mybir.AxisListType.X,
                op=mybir.AluOpType.add,
            )

    pf = pG.tile([1, 32], _F32, tag="cs")
    nc.tensor.matmul(pf[:], ones[:], staging[:], start=True, stop=True)
    osb = const.tile([1, 32], _F32)
    nc.vector.tensor_copy(osb[:], pf[:])
    nc.sync.dma_start(out[:, :], osb[:, :])


def build():
    if "nc" in _NC_CACHE:
        return _NC_CACHE["nc"]
    nc = bacc.Bacc(
        "TRN2",
        target_bir_lowering=False,
        debug=False,
        enable_asserts=False,
        num_devices=N_CORES,
    )
    feat = nc.dram_tensor("feat", [PR, D], _FP8, kind="ExternalInput").ap()
    cnt = nc.dram_tensor("cnt", [128, 16], _F32, kind="ExternalInput").ap()
    oh = nc.dram_tensor("oh", [128, NB * SUB, 128], _FP8, kind="ExternalInput").ap()
    cbf = nc.dram_tensor("ctab", [128 * NB, D], _FP8, kind="ExternalInput").ap()
    nbf = nc.dram_tensor("ntab", [128 * NB, D], _FP8, kind="ExternalInput").ap()
    out = nc.dram_tensor("out", [1, 32], _F32, kind="ExternalOutput").ap()
    with tile.TileContext(nc) as tc, ExitStack() as ctx:
        _build_body(ctx, tc, feat, cnt, oh, cbf, nbf, out)
    nc.compile()
    _NC_CACHE["nc"] = nc
    return nc


def _bank_assignment(y):
    """Greedy-balanced partition of the C classes into NB banks of CPB each."""
    counts = np.bincount(y, minlength=C)
    order = np.argsort(-counts, kind="stable")
    bank_tot = np.zeros(NB, dtype=np.int64)
    bank_n = np.zeros(NB, dtype=np.int64)
    bankclasses = np.zeros((NB, CPB), dtype=np.int64)
    cls_bank = np.zeros(C, dtype=np.int64)
    cls_pos = np.zeros(C, dtype=np.int64)
    for c in order:
        open_banks = np.flatnonzero(bank_n < CPB)
        j = open_banks[np.argmin(bank_tot[open_banks])]
        bankclasses[j, bank_n[j]] = c
        cls_bank[c] = j
        cls_pos[c] = bank_n[j]
        bank_n[j] += 1
        bank_tot[j] += counts[c]
    assert bank_tot.max() <= GROUP, f"bank overflow: {bank_tot.max()} > {GROUP}"
    return bankclasses, cls_bank, cls_pos, counts


def make_in_maps(y, feat, centers):
    feat = np.ascontiguousarray(feat, dtype=np.float32)
    centers = np.ascontiguousarray(centers, dtype=np.float32)
    y = np.asarray(y).astype(np.int64)
    norm2 = np.sum(centers.astype(np.float64) ** 2, axis=1, keepdims=True)
    ncenters = (centers / np.sqrt(norm2)).astype(ml_dtypes.float8_e4m3)
    in_maps = []
    for i in range(N_CORES):
        ys = y[i * BS : (i + 1) * BS]
        fs = feat[i * BS : (i + 1) * BS]
        bankclasses, cls_bank, cls_pos, counts = _bank_assignment(ys)

        # bank-major padded tables: dram row r (r%128 < 125) = class
        # bankclasses[r // 128][r % 128]
        ctab = np.zeros((128 * NB, D), dtype=ml_dtypes.float8_e4m3)
        ntab = np.zeros((128 * NB, D), dtype=ml_dtypes.float8_e4m3)
        rr = np.arange(128 * NB)
        vr = rr % 128 < CPB
        src = bankclasses[rr[vr] // 128, rr[vr] % 128]
        ctab[vr] = centers[src].astype(ml_dtypes.float8_e4m3)
        ntab[vr] = ncenters[src]

        # bucket rows by bank; group g rows sit at slots [g*GROUP, g*GROUP+n_g)
        row_bank = cls_bank[ys]
        grp_order = np.argsort(row_bank, kind="stable")
        n_per = np.bincount(row_bank, minlength=NB)
        starts = np.zeros(NB + 1, dtype=np.int64)
        starts[1:] = np.cumsum(n_per)
        slot = np.full(PR, -1, dtype=np.int64)  # slot -> source row
        for g in range(NB):
            rows = grp_order[starts[g] : starts[g + 1]]
            slot[g * GROUP : g * GROUP + len(rows)] = rows

        featp = np.zeros((PR, D), dtype=ml_dtypes.float8_e4m3)
        valid = slot >= 0
        featp[valid] = fs[slot[valid]].astype(ml_dtypes.float8_e4m3)

        # onehot for matmul (g, s): row at (part p, slot s) is padded row
        # g*GROUP + 9p + s; pads get no column
        oh = np.zeros((128, NB * SUB, 128), dtype=ml_dtypes.float8_e4m3)
        k = np.flatnonzero(valid)
        g_k = k // GROUP
        r_k = k % GROUP
        p_k = r_k // SUB
        s_k = r_k % SUB
        oh[p_k, g_k * SUB + s_k, cls_pos[ys[slot[k]]]] = 1.0

        cnt_pb = np.zeros((128, 16), dtype=np.float32)
        cnt_pb[cls_pos, cls_bank] = counts
        cnt_pb[cls_pos, NB + cls_bank] = norm2[:, 0]

        in_maps.append(
            {
                "feat": featp,
                "cnt": cnt_pb,
                "oh": oh,
                "ctab": ctab,
                "ntab": ntab,
            }
        )
    return in_maps


def combine(outs):
    """outs: list of 8 [1,32] f32 arrays -> scalar loss (np.float32)."""
    cen = 0.0
    for o in outs:
        o = np.asarray(o, dtype=np.float64)
        cen += o[0, 0:9].sum() + o[0, 16:24].sum() - 2.0 * o[0, 10:14].sum()
    o0 = np.asarray(outs[0], dtype=np.float64)
    gsq, ssq = o0[0, 9] + o0[0, 15], o0[0, 14]
    ang = gsq - 2.0 * CT * ssq + C * C * CT * CT - C * (1.0 - CT) ** 2
    loss = 0.5 * cen / B + ang / (0.5 * C * (C - 1))
    return np.float32(loss)


def kernel(y, feat, centers):
    nc = build()
    in_maps = make_in_maps(y, feat, centers)
    res = run_bass_kernel_spmd(nc, in_maps, core_ids=list(range(N_CORES)))
    return combine([res.results[i]["out"] for i in range(N_CORES)])


# revision 41
# speedup vs baseline: 1.2019x; 1.2019x over previous
"""Trainium2 Bass kernel for AngelLoss (center loss + angular loss).

loss = 0.5*sum((feat - centers[y])^2)/B
     + sum_offdiag((c_i.c_j/(|c_i||c_j|) - ct)^2) / (0.5*C*(C-1))

Sharding (8 NeuronCores, data-parallel over batch):
  - center term, gather-free:  sum||f||^2 - 2*sum_c c_c.S_c + sum_c n_c||c_c||^2
    where S_c = sum of feat rows with label c.  Host buckets each core's
    8192 rows into 8 class-banks (125 classes each, greedy-balanced), pads
    each bank's rows to 9x128 and ships them fp8 along with fp8 onehots.
    S for two banks accumulates in one 2-bank PSUM tile via 2x9
    onehot^T @ feat fp8 matmuls and drains with one DVE multiply +
    free-axis reduce against the resident bf16 centers.  sum||f||^2
    splits 7/2 across ScalarE (square+accum) and DVE (mult+reduce).
  - angular term via the Frobenius identity (N = row-normalized centers):
      sum_ij (sim-ct)^2 = ||N^T N||_F^2 - 2ct ||sum_i N_i||^2 + C^2 ct^2
    computed redundantly on every core from the resident fp8 normalized
    table with 2x16 accumulating matmuls into 2-bank PSUM Gram halves.
  - per-core [1,32] partial sums are combined on the host.
"""

from contextlib import ExitStack

import ml_dtypes
import numpy as np

import concourse.bass as bass
import concourse.tile as tile
from concourse import bacc, mybir
from concourse.bass import ds, ts
from concourse.bass_utils import run_bass_kernel_spmd

N_CORES = 8
B, C, D = 65536, 1000, 512
BS = B // N_CORES  # 8192 rows per core
NB = 8  # class banks
CPB = C // NB  # 125 classes per bank
SUB = 9  # 128-row subtiles per bank group (1152 slots >= ~1024+slack rows)
GROUP = 128 * SUB  # 1152
PR = NB * GROUP  # 9216 padded rows per core
NSC = 7  # feat slots squared on ScalarE; the rest (SUB-NSC) go to DVE

# ct = 2*radius(C-1)^2 - 1 from the reference, evaluated in f64, cast f32.
CT = float(np.float32(-0.0010010010010047532))

_F32 = mybir.dt.float32
_BF16 = mybir.dt.bfloat16
_FP8 = mybir.dt.float8e4

_NC_CACHE = {}


def _build_body(ctx, tc, feat, cnt, oh, cbf, nbf, out):
    nc = tc.nc
    AF = mybir.ActivationFunctionType

    const = ctx.enter_context(tc.tile_pool(name="const", bufs=1))
    pnrm = ctx.enter_context(tc.tile_pool(name="nrm", bufs=2))
    pfeat = ctx.enter_context(tc.tile_pool(name="feat", bufs=3))
    psq = ctx.enter_context(tc.tile_pool(name="sq", bufs=2))
    pdscr = ctx.enter_context(tc.tile_pool(name="dscr", bufs=2))
    pG = ctx.enter_context(tc.tile_pool(name="G", bufs=1, space="PSUM"))
    pS = ctx.enter_context(tc.tile_pool(name="S", bufs=2, space="PSUM"))

    # scalar queue: onehots + counts; gpsimd queue: the two tables;
    # sync queue: the feat stream (nothing else competes with it)
    oht = const.tile([128, NB * SUB, 128], _FP8)
    ct_all = const.tile([128, NB, D], _FP8)
    nt_all = const.tile([128, NB, D], _FP8)
    cntt = const.tile([128, 16], _F32)
    # everything but the feat stream issues on the otherwise-idle gpsimd
    # ring, ordered by when the consumer needs it
    nc.gpsimd.dma_start(oht[:, ds(0, 2 * SUB), :], oh[:, ds(0, 2 * SUB), :])
    nc.gpsimd.dma_start(ct_all[:], cbf.rearrange("(p s) d -> p s d", p=128))
    nc.gpsimd.dma_start(oht[:, ds(2 * SUB, 2 * SUB), :], oh[:, ds(2 * SUB, 2 * SUB), :])
    nc.gpsimd.dma_start(nt_all[:], nbf.rearrange("(p s) d -> p s d", p=128))
    nc.gpsimd.dma_start(oht[:, ds(4 * SUB, 2 * SUB), :], oh[:, ds(4 * SUB, 2 * SUB), :])
    nc.gpsimd.dma_start(oht[:, ds(6 * SUB, 2 * SUB), :], oh[:, ds(6 * SUB, 2 * SUB), :])
    nc.gpsimd.dma_start(cntt[:], cnt[:, :])

    ones = const.tile([128, 1], _F32)
    nc.vector.memset(ones[:], 1.0)
    onesp2 = const.tile([128, 2, 1], _FP8)
    nc.vector.memset(onesp2[:], 1.0)
    # staging cols: 0-7 scalar sum(f^2); 8 counts.|c|^2; 9,15 ||G||^2
    # halves; 10-13 +cross per bank pair (host applies -2); 14 ||colsum||^2;
    # 16-23 vector sum(f^2).
    staging = const.tile([128, 32], _F32)
    nc.vector.memset(staging[:], 0.0)

    # --- center-loss main loop; S PSUM tile covers two banks ---
    for g in range(NB):
        if g % 2 == 0:
            st = pS.tile([128, 2, D], _F32, tag="S")
        ft = pfeat.tile([128, SUB, D], _FP8, tag="ft")
        nc.sync.dma_start(
            ft[:], feat[ds(g * GROUP, GROUP), :].rearrange("(p s) d -> p s d", p=128)
        )
        sqs = psq.tile([128, NSC, D], _FP8, tag="sqs")
        nc.scalar.activation(
            sqs[:], ft[:, :NSC, :], AF.Square, accum_out=staging[:, g : g + 1]
        )
        sqv = psq.tile([128, SUB - NSC, D], _FP8, tag="sqv")
        nc.vector.tensor_tensor(
            out=sqv[:],
            in0=ft[:, NSC:, :],
            in1=ft[:, NSC:, :],
            op=mybir.AluOpType.mult,
        )
        nc.vector.tensor_reduce(
            out=staging[:, 16 + g : 17 + g],
            in_=sqv[:],
            axis=mybir.AxisListType.XY,
            op=mybir.AluOpType.add,
        )
        for sp in range(0, SUB - 1, 2):
            nc.tensor.matmul(
                st[:, g % 2, :],
                oht[:, ds(g * SUB + sp, 2), :],
                ft[:, ds(sp, 2), :],
                start=(sp == 0),
                stop=False,
                perf_mode=mybir.MatmulPerfMode.DoubleRow,
            )
        nc.tensor.matmul(
            st[:, g % 2, :],
            oht[:, g * SUB + SUB - 1, :],
            ft[:, SUB - 1, :],
            start=False,
            stop=True,
        )
        if g % 2 == 1:
            dscr = pdscr.tile([CPB, 2, D], _F32, tag="dscr")
            nc.vector.tensor_tensor(
                out=dscr[:],
                in0=st[:CPB, :, :],
                in1=ct_all[:CPB, ds(g - 1, 2), :],
                op=mybir.AluOpType.mult,
            )
            nc.vector.tensor_reduce(
                out=staging[:CPB, 10 + g // 2 : 11 + g // 2],
                in_=dscr[:],
                axis=mybir.AxisListType.XY,
                op=mybir.AluOpType.add,
            )
        if g in (2, 4):
            # angular Gram (two 2-bank passes) slots into the Tensor
            # stream here: the normalized table is resident and the
            # scatter stream has slack
            kis = (0, 1) if g == 2 else (2, 3)
            Gt = pG.tile([128, 2, D], _F32, tag="G")
            for jp in range(0, NB, 2):
                for kx, ki in enumerate(kis):
                    nc.tensor.matmul(
                        Gt[:, kx, :],
                        nt_all[:CPB, ds(jp, 2), ts(ki, 128)],
                        nt_all[:CPB, ds(jp, 2), :],
                        start=(jp == 0),
                        stop=(jp == NB - 2),
                        perf_mode=mybir.MatmulPerfMode.DoubleRow,
                    )
            col = 9 if g == 2 else 15
            gsq = pnrm.tile([128, 2, D], _F32, tag="gsq")
            nc.scalar.activation(
                gsq[:], Gt[:], AF.Square, accum_out=staging[:, col : col + 1]
            )
        if g == 3:
            # colsum of the normalized table on the idle cs PSUM bank
            csf = pG.tile([1, D], _F32, tag="cs")
            for j in range(NB):
                nc.tensor.matmul(
                    csf[0:1, :],
                    onesp2[:CPB, 0, :],
                    nt_all[:CPB, j, :],
                    start=(j == 0),
                    stop=(j == NB - 1),
                )
            css = pnrm.tile([1, D], _F32, tag="css")
            nc.scalar.activation(
                css[0:1, :], csf[0:1, :], AF.Square, accum_out=staging[0:1, 14:15]
            )
        if g == 4:
            # counts . |c|^2 (norms^2 host-packed beside the counts)
            cscr = pnrm.tile([CPB, NB], _F32, tag="cscr")
            nc.vector.tensor_tensor(
                out=cscr[:],
                in0=cntt[:CPB, 0:NB],
                in1=cntt[:CPB, NB:16],
                op=mybir.AluOpType.mult,
            )
            nc.vector.tensor_reduce(
                out=staging[:CPB, 8:9],
                in_=cscr[:],
                axis=mybir.AxisListType.X,
                op=mybir.AluOpType.add,
            )

    pf = pG.tile([1, 32], _F32, tag="cs")
    nc.tensor.matmul(pf[:], ones[:], staging[:], start=True, stop=True)
    osb = const.tile([1, 32], _F32)
    nc.vector.tensor_copy(osb[:], pf[:])
    nc.sync.dma_start(out[:, :], osb[:, :])


def build():
    if "nc" in _NC_CACHE:
        return _NC_CACHE["nc"]
    nc = bacc.Bacc(
        "TRN2",
        target_bir_lowering=False,
        debug=False,
        enable_asserts=False,
        num_devices=N_CORES,
    )
    feat = nc.dram_tensor("feat", [PR, D], _FP8, kind="ExternalInput").ap()
    cnt = nc.dram_tensor("cnt", [128, 16], _F32, kind="ExternalInput").ap()
    oh = nc.dram_tensor("oh", [128, NB * SUB, 128], _FP8, kind="ExternalInput").ap()
    cbf = nc.dram_tensor("ctab", [128 * NB, D], _FP8, kind="ExternalInput").ap()
    nbf = nc.dram_tensor("ntab", [128 * NB, D], _FP8, kind="ExternalInput").ap()
    out = nc.dram_tensor("out", [1, 32], _F32, kind="ExternalOutput").ap()
    with tile.TileContext(nc) as tc, ExitStack() as ctx:
        _build_body(ctx, tc, feat, cnt, oh, cbf, nbf, out)
    nc.compile()
    _NC_CACHE["nc"] = nc
    return nc


def _bank_assignment(y):
    """Greedy-balanced partition of the C classes into NB banks of CPB each."""
    counts = np.bincount(y, minlength=C)
    order = np.argsort(-counts, kind="stable")
    bank_tot = np.zeros(NB, dtype=np.int64)
    bank_n = np.zeros(NB, dtype=np.int64)
    bankclasses = np.zeros((NB, CPB), dtype=np.int64)
    cls_bank = np.zeros(C, dtype=np.int64)
    cls_pos = np.zeros(C, dtype=np.int64)
    for c in order:
        open_banks = np.flatnonzero(bank_n < CPB)
        j = open_banks[np.argmin(bank_tot[open_banks])]
        bankclasses[j, bank_n[j]] = c
        cls_bank[c] = j
        cls_pos[c] = bank_n[j]
        bank_n[j] += 1
        bank_tot[j] += counts[c]
    assert bank_tot.max() <= GROUP, f"bank overflow: {bank_tot.max()} > {GROUP}"
    return bankclasses, cls_bank, cls_pos, counts


def make_in_maps(y, feat, centers):
    feat = np.ascontiguousarray(feat, dtype=np.float32)
    centers = np.ascontiguousarray(centers, dtype=np.float32)
    y = np.asarray(y).astype(np.int64)
    norm2 = np.sum(centers.astype(np.float64) ** 2, axis=1, keepdims=True)
    ncenters = (centers / np.sqrt(norm2)).astype(ml_dtypes.float8_e4m3)
    in_maps = []
    for i in range(N_CORES):
        ys = y[i * BS : (i + 1) * BS]
        fs = feat[i * BS : (i + 1) * BS]
        bankclasses, cls_bank, cls_pos, counts = _bank_assignment(ys)

        # bank-major padded tables: dram row r (r%128 < 125) = class
        # bankclasses[r // 128][r % 128]
        ctab = np.zeros((128 * NB, D), dtype=ml_dtypes.float8_e4m3)
        ntab = np.zeros((128 * NB, D), dtype=ml_dtypes.float8_e4m3)
        rr = np.arange(128 * NB)
        vr = rr % 128 < CPB
        src = bankclasses[rr[vr] // 128, rr[vr] % 128]
        ctab[vr] = centers[src].astype(ml_dtypes.float8_e4m3)
        ntab[vr] = ncenters[src]

        # bucket rows by bank; group g rows sit at slots [g*GROUP, g*GROUP+n_g)
        row_bank = cls_bank[ys]
        grp_order = np.argsort(row_bank, kind="stable")
        n_per = np.bincount(row_bank, minlength=NB)
        starts = np.zeros(NB + 1, dtype=np.int64)
        starts[1:] = np.cumsum(n_per)
        slot = np.full(PR, -1, dtype=np.int64)  # slot -> source row
        for g in range(NB):
            rows = grp_order[starts[g] : starts[g + 1]]
            slot[g * GROUP : g * GROUP + len(rows)] = rows

        featp = np.zeros((PR, D), dtype=ml_dtypes.float8_e4m3)
        valid = slot >= 0
        featp[valid] = fs[slot[valid]].astype(ml_dtypes.float8_e4m3)

        # onehot for matmul (g, s): row at (part p, slot s) is padded row
        # g*GROUP + 9p + s; pads get no column
        oh = np.zeros((128, NB * SUB, 128), dtype=ml_dtypes.float8_e4m3)
        k = np.flatnonzero(valid)
        g_k = k // GROUP
        r_k = k % GROUP
        p_k = r_k // SUB
        s_k = r_k % SUB
        oh[p_k, g_k * SUB + s_k, cls_pos[ys[slot[k]]]] = 1.0

        cnt_pb = np.zeros((128, 16), dtype=np.float32)
        cnt_pb[cls_pos, cls_bank] = counts
        cnt_pb[cls_pos, NB + cls_bank] = norm2[:, 0]

        in_maps.append(
            {
                "feat": featp,
                "cnt": cnt_pb,
                "oh": oh,
                "ctab": ctab,
                "ntab": ntab,
            }
        )
    return in_maps


def combine(outs):
    """outs: list of 8 [1,32] f32 arrays -> scalar loss (np.float32)."""
    cen = 0.0
    for o in outs:
        o = np.asarray(o, dtype=np.float64)
        cen += o[0, 0:9].sum() + o[0, 16:24].sum() - 2.0 * o[0, 10:14].sum()
    o0 = np.asarray(outs[0], dtype=np.float64)
    gsq, ssq = o0[0, 9] + o0[0, 15], o0[0, 14]
    ang = gsq - 2.0 * CT * ssq + C * C * CT * CT - C * (1.0 - CT) ** 2
    loss = 0.5 * cen / B + ang / (0.5 * C * (C - 1))
    return np.float32(loss)


def kernel(y, feat, centers):
    nc = build()
    in_maps = make_in_maps(y, feat, centers)
    res = run_bass_kernel_spmd(nc, in_maps, core_ids=list(range(N_CORES)))
    return combine([res.results[i]["out"] for i in range(N_CORES)])
